# revision 11
# baseline (speedup 1.0000x reference)
"""Trainium2 Bass kernel for CrossShotTransitionHamiltonian.

Math: H = H_idx (x) I_64 with H_idx the 16x16 cycle adjacency matrix, so
U_b = exp(-lam_b H) = M_b (x) I_64 where M_b = expm(-lam_b * H_idx) is a
16x16 symmetric matrix computed exactly on the host from the (tiny) batch
scalars lam_b.  The heavy device work per batch element is the congruence
rho_out = A rho A (A = M (x) I_64, all symmetric) plus trace normalization
(trace folded into the stage-2 operand on the host).

Device algorithm per batch (1024x1024), per core (4 batches/core):
  - rho arrives pre-permuted by the host so partition p = a*16 + k holds
    rows k*64 + a*8 + q and the free dim holds columns in the order
    f = q*1024 + u*128 + blo*32 + (l*2 + bhi)   (c = l*64 + bhi*32 +
    blo*8 + u).  A then acts as a dense 128x128 stationary operand:
    Z = A @ rho (16 matmuls, fp32 PSUM), and every PSUM->SBUF copy is a
    plain contiguous [128,1024] cast copy.
  - ONE DVE stream-transpose instruction (independent 32x32 block
    transposes) moves the contraction index l onto partitions:
    zt[32P + l*2 + bhi, ...].  The 32-block row-group P that lands on the
    contraction axis is neutralized by a modified stage-2 operator
        kron2p[P*32 + l*2 + bhi, j*8 + bhi*4 + P] = M[l,j] / trace
    (block-diagonal over (P, bhi), contracts only l) -- so no PE
    transposes and no strided engine access patterns anywhere.
  - Stage 2: Y = (A/trace) @ Z^T (16 matmuls), contiguous copies, one
    flat contiguous DMA out.  The host un-permutes the result (out is
    symmetric, orientation is irrelevant).

The whole pipeline runs in bf16 (fp32 PSUM accumulation); measured
end-to-end rel err ~3.4e-3 vs the 2e-2 gate.  PSUM->SBUF copies run 2:1
on Activation vs DVE (DVE also owns the stream transposes).

Data-parallel over batch across 8 NeuronCores, no collectives.
"""

import numpy as np

from concourse import bacc, mybir
from concourse import tile
from concourse.bass_utils import run_bass_kernel_spmd

NB = 4  # batch elements per core
NCORES = 8
D = 1024
F32 = mybir.dt.float32
BF16 = mybir.dt.bfloat16


def _build_body(nc, tc, rho_d, kron_d, kron2_d, out_d, nb=NB, reps=1):
    from contextlib import ExitStack

    with ExitStack() as ctx:
        pool = ctx.enter_context(tc.tile_pool(name="work", bufs=1))
        pp = ctx.enter_context(tc.tile_pool(name="ps", bufs=1, space="PSUM"))

        def copy_engine(n):
            # 3:1 rotation ACT:DVE (DVE also runs the stream transposes)
            return nc.scalar.copy if n % 4 != 3 else nc.vector.tensor_copy

        ncopy = 0
        for r in range(reps):
            for i in range(nb):
                u = f"{r}_{i}"
                zin = pool.tile([128, 8192], BF16, tag="zin", bufs=2, name=f"zin{u}")
                nc.sync.dma_start(out=zin[:], in_=rho_d[i])
                kr = pool.tile([128, 128], BF16, tag="kr", bufs=2, name=f"kr{u}")
                nc.scalar.dma_start(out=kr[:], in_=kron_d[i])
                kr2 = pool.tile([128, 128], BF16, tag="kr2", bufs=2, name=f"kr2{u}")
                nc.scalar.dma_start(out=kr2[:], in_=kron2_d[i])

                # ---------- stage 1: Z = A @ rho ----------
                zsb = pool.tile([128, 8192], BF16, tag="zsb", bufs=2, name=f"zsb{u}")
                for q in range(4):
                    pz = pp.tile([128, 2048], F32, tag="pmm", bufs=2, name=f"pz{u}_{q}")
                    for h in range(4):
                        sl = slice(2048 * q + 512 * h, 2048 * q + 512 * (h + 1))
                        nc.tensor.matmul(
                            pz[:, 512 * h : 512 * (h + 1)],
                            lhsT=kr[:],
                            rhs=zin[:, sl],
                            start=True, stop=True,
                        )
                    copy_engine(ncopy)(
                        out=zsb[:, 2048 * q : 2048 * (q + 1)], in_=pz[:]
                    )
                    ncopy += 1

                # ---------- DVE 32x32 block transpose: Z -> zt ----------
                zt = pool.tile([128, 8192], BF16, tag="zt", bufs=2, name=f"zt{u}")
                nc.vector.transpose(out=zt[:, :4096], in_=zsb[:, :4096])
                nc.vector.transpose(out=zt[:, 4096:], in_=zsb[:, 4096:])

                # ---------- stage 2: Y = (A/trace) @ Z^T ----------
                ysb = pool.tile([128, 8192], BF16, tag="ysb", bufs=2, name=f"ysb{u}")
                for t in range(4):
                    po = pp.tile([128, 2048], F32, tag="pmm", bufs=2, name=f"po{u}_{t}")
                    for h in range(4):
                        sl = slice(2048 * t + 512 * h, 2048 * t + 512 * (h + 1))
                        nc.tensor.matmul(
                            po[:, 512 * h : 512 * (h + 1)],
                            lhsT=kr2[:],
                            rhs=zt[:, sl],
                            start=True, stop=True,
                        )
                    copy_engine(ncopy)(
                        out=ysb[:, 2048 * t : 2048 * (t + 1)], in_=po[:]
                    )
                    ncopy += 1

                nc.scalar.dma_start(out=out_d[i], in_=ysb[:])


def build_nc(nb=NB, reps=1):
    nc = bacc.Bacc(
        "TRN2",
        target_bir_lowering=False,
        debug=False,
        enable_asserts=False,
        num_devices=NCORES,
    )
    rho_d = nc.dram_tensor("rho", (nb, 128, 8192), BF16, kind="ExternalInput").ap()
    kron_d = nc.dram_tensor("kron", (nb, 128, 128), BF16, kind="ExternalInput").ap()
    kron2_d = nc.dram_tensor("kron2", (nb, 128, 128), BF16, kind="ExternalInput").ap()
    out_d = nc.dram_tensor("out", (nb, 128, 8192), BF16, kind="ExternalOutput").ap()

    with tile.TileContext(nc) as tc:
        _build_body(nc, tc, rho_d, kron_d, kron2_d, out_d, nb=nb, reps=reps)
    nc.compile()
    return nc


# ---------------- host-side parameter prep ----------------

def _bf16(x):
    import ml_dtypes

    return np.asarray(x, dtype=np.float32).astype(ml_dtypes.bfloat16)


def _index_maps():
    """Flat gather indices: IN maps device rho layout -> natural rho; OUT
    maps device out layout -> natural out positions."""
    if "idx" in _CACHE:
        return _CACHE["idx"]
    # input: position (p=a*16+k, q, f_c=(u,blo,l,bhi)) <- rho[row, col]
    a, k, q = np.meshgrid(np.arange(8), np.arange(16), np.arange(8), indexing="ij")
    row = (k * 64 + a * 8 + q).reshape(128, 8)  # [p, q]
    u2, blo, l, bhi = np.meshgrid(
        np.arange(8), np.arange(4), np.arange(16), np.arange(2), indexing="ij"
    )
    col = (l * 64 + bhi * 32 + blo * 8 + u2).reshape(1024)  # [f_c]
    in_idx = (row[:, :, None] * 1024 + col[None, None, :]).reshape(-1)  # (128*8192,)

    # output: position (y=j*8+bhi*4+P, f=q*1024+u*128+blo*32+v) -> out[c2, r]
    j, bhi2, P = np.meshgrid(np.arange(16), np.arange(2), np.arange(4), indexing="ij")
    j, bhi2, P = j.reshape(128), bhi2.reshape(128), P.reshape(128)  # [y]
    q2, u3, blo2, v = np.meshgrid(
        np.arange(8), np.arange(8), np.arange(4), np.arange(32), indexing="ij"
    )
    q2, u3, blo2, v = (x.reshape(8192) for x in (q2, u3, blo2, v))  # [f]
    c2 = j[:, None] * 64 + bhi2[:, None] * 32 + blo2[None, :] * 8 + u3[None, :]
    rr = P[:, None] * 256 + v[None, :] * 8 + q2[None, :]
    out_idx = (c2 * 1024 + rr).reshape(-1)  # (128*8192,)
    _CACHE["idx"] = (in_idx, out_idx)
    return in_idx, out_idx


def _host_params(t, w1, b1, w2, b2):
    x = t.astype(np.float64)[:, None]
    h = x @ w1.astype(np.float64).T + b1.astype(np.float64)
    h = h / (1.0 + np.exp(-h))  # silu
    lam = 0.1 * np.tanh(h @ w2.astype(np.float64).T + b2.astype(np.float64))[:, 0]

    k = np.arange(16)
    S = np.zeros((16, 16))
    S[(k + 1) % 16, k] = 1.0
    Hidx = S + S.T
    w_eig, V = np.linalg.eigh(Hidx)
    E = np.exp(-lam[:, None] * w_eig[None, :])  # (B,16)
    M = np.einsum("ik,bk,jk->bij", V, E, V)  # (B,16,16)

    B = M.shape[0]
    # stage-1 lhsT: in-partitions a-major (p = a*16 + k), out (m = i*8 + a):
    # kron1[b, a*16+k, i*8+a] = M[b, i, k]
    kron1 = np.zeros((B, 8, 16, 16, 8))
    for a_sub in range(8):
        kron1[:, a_sub, :, :, a_sub] = np.transpose(M, (0, 2, 1))
    kron = np.ascontiguousarray(kron1.reshape(B, 128, 128), dtype=np.float32)
    # stage-2 lhsT for the 32x32-block-transposed zt:
    # kron2p[b, P*32 + l*2 + bhi, j*8 + bhi*4 + P] = M[b, l, j]
    kron2p = np.zeros((B, 4, 16, 2, 16, 2, 4))
    for P in range(4):
        for bhi in range(2):
            kron2p[:, P, :, bhi, :, bhi, P] = M
    kron2 = np.ascontiguousarray(kron2p.reshape(B, 128, 128), dtype=np.float32)
    return kron, kron2


_CACHE = {}


def _host_traces(rho, t, w1, b1, w2, b2):
    """tr(A^2 rho) per batch from rho's block diagonals (tiny: 0.5M MACs)."""
    x = t.astype(np.float64)[:, None]
    h = x @ w1.astype(np.float64).T + b1.astype(np.float64)
    h = h / (1.0 + np.exp(-h))
    lam = 0.1 * np.tanh(h @ w2.astype(np.float64).T + b2.astype(np.float64))[:, 0]
    k = np.arange(16)
    S = np.zeros((16, 16))
    S[(k + 1) % 16, k] = 1.0
    w_eig, V = np.linalg.eigh(S + S.T)
    E = np.exp(-lam[:, None] * w_eig[None, :])
    M = np.einsum("ik,bk,jk->bij", V, E, V)
    M2 = np.einsum("bij,bjk->bik", M, M)
    rr = rho.reshape(rho.shape[0], 16, 64, 16, 64)
    c = np.einsum("bkala->bkl", rr, optimize=True)
    return np.einsum("bkl,bkl->b", c.astype(np.float64), M2)


def _prep_in_maps(rho, t, w1, b1, w2, b2):
    rho = np.ascontiguousarray(rho, dtype=np.float32)
    kron, kron2 = _host_params(
        np.asarray(t), np.asarray(w1), np.asarray(b1), np.asarray(w2), np.asarray(b2)
    )
    tr = _host_traces(rho, np.asarray(t), np.asarray(w1), np.asarray(b1),
                      np.asarray(w2), np.asarray(b2))
    kron2 = kron2 / np.maximum(tr, 1e-8)[:, None, None]
    in_idx, _ = _index_maps()
    B = rho.shape[0]
    rho_p = np.take(rho.reshape(B, -1), in_idx, axis=1).reshape(B, 128, 8192)
    rho_b = _bf16(rho_p)
    kron_b = _bf16(kron)
    kron2_b = _bf16(kron2)

    in_maps = []
    for c in range(NCORES):
        sl = slice(NB * c, NB * (c + 1))
        in_maps.append(
            {
                "rho": np.ascontiguousarray(rho_b[sl]),
                "kron": np.ascontiguousarray(kron_b[sl]),
                "kron2": np.ascontiguousarray(kron2_b[sl]),
            }
        )
    return in_maps


def kernel(rho, t, w1, b1, w2, b2, H):
    in_maps = _prep_in_maps(rho, t, w1, b1, w2, b2)
    if "nc" not in _CACHE:
        _CACHE["nc"] = build_nc()
    nc = _CACHE["nc"]

    last_err = None
    for attempt in range(3):
        try:
            res = run_bass_kernel_spmd(nc, in_maps, core_ids=list(range(NCORES)))
            break
        except Exception as e:  # transient device-unrecoverable faults heal on retry
            last_err = e
            import time as _time

            _time.sleep(5.0)
    else:
        raise last_err
    dev = np.concatenate(
        [np.asarray(res.results[c]["out"], dtype=np.float32) for c in range(NCORES)],
        axis=0,
    )
    _, out_idx = _index_maps()
    B = dev.shape[0]
    out = np.empty((B, D * D), dtype=np.float32)
    out[:, out_idx] = dev.reshape(B, -1)
    return out.reshape(B, D, D)


def _make_runner(nc, in_maps):
    """Build a jitted dispatcher for nc; returns fn() -> wall ns per call."""
    import time
    import jax
    import jax.numpy as jnp
    from jax.experimental.shard_map import shard_map
    from jax.sharding import Mesh, NamedSharding, PartitionSpec

    from concourse.bass2jax import _bass_exec_p, install_neuronx_cc_hook
    from concourse.bass2jax import partition_id_tensor

    install_neuronx_cc_hook()
    part_name = nc.partition_id_tensor.name if nc.partition_id_tensor else None
    in_names, out_names, out_avals, zero_outs = [], [], [], []
    for alloc in nc.m.functions[0].allocations:
        if not isinstance(alloc, mybir.MemoryLocationSet):
            continue
        name = alloc.memorylocations[0].name
        if alloc.kind == "ExternalInput":
            if name != part_name:
                in_names.append(name)
        elif alloc.kind == "ExternalOutput":
            out_names.append(name)
            shape = tuple(alloc.tensor_shape)
            dtype = mybir.dt.np(alloc.dtype)
            out_avals.append(jax.core.ShapedArray(shape, dtype))
            zero_outs.append((shape, dtype))
    n_params = len(in_names)
    n_outs = len(out_avals)
    all_names = in_names + out_names
    if part_name is not None:
        all_names = all_names + [part_name]
    donate = tuple(range(n_params, n_params + n_outs))

    def _body(*args):
        operands = list(args)
        if part_name is not None:
            operands.append(partition_id_tensor())
        outs = _bass_exec_p.bind(
            *operands,
            out_avals=tuple(out_avals),
            in_names=tuple(all_names),
            out_names=tuple(out_names),
            lowering_input_output_aliases=(),
            sim_require_finite=True,
            sim_require_nnan=True,
            nc=nc,
        )
        return tuple(outs)

    devices = jax.devices()[:NCORES]
    mesh = Mesh(np.asarray(devices), ("core",))
    in_specs = (PartitionSpec("core"),) * (n_params + n_outs)
    out_specs = (PartitionSpec("core"),) * n_outs
    sharded = jax.jit(
        shard_map(_body, mesh=mesh, in_specs=in_specs, out_specs=out_specs,
                  check_rep=False),
        donate_argnums=donate,
        keep_unused=True,
    )
    sh = NamedSharding(mesh, PartitionSpec("core"))
    concat_in = [
        jax.device_put(
            np.concatenate([np.asarray(in_maps[c][n])[None] for c in range(NCORES)],
                           axis=0).reshape((-1, *np.asarray(in_maps[0][n]).shape[1:])),
            sh,
        )
        for n in in_names
    ]
    mkz = jax.jit(
        lambda: tuple(
            jnp.zeros((NCORES * s[0], *s[1:]), d) for (s, d) in zero_outs
        ),
        out_shardings=tuple(sh for _ in zero_outs),
    )

    def run():
        zs = mkz()
        jax.block_until_ready(zs)
        t0 = time.perf_counter()
        out = sharded(*concat_in, *zs)
        jax.block_until_ready(out)
        t1 = time.perf_counter()
        return (t1 - t0) * 1e9

    return run


def timed_runs(inputs, iters=10, nc=None):
    """Wall times (ns) for repeated dispatches of one NEFF."""
    in_maps = _prep_in_maps(
        inputs["rho"], inputs["t"], inputs["w1"], inputs["b1"],
        inputs["w2"], inputs["b2"],
    )
    if nc is None:
        if "nc" not in _CACHE:
            _CACHE["nc"] = build_nc()
        nc = _CACHE["nc"]
    run = _make_runner(nc, in_maps)
    run()  # warm-up / compile
    return [run() for _ in range(iters)]


def timed_pairs(inputs, reps, iters=14):
    """Interleave 1-rep and reps-rep NEFF dispatches; the paired difference
    cancels the slow drift of the RPC dispatch floor.  Returns per-pair
    marginal device ns/exec list: (tR - t1) / (reps - 1)."""
    in_maps = _prep_in_maps(
        inputs["rho"], inputs["t"], inputs["w1"], inputs["b1"],
        inputs["w2"], inputs["b2"],
    )
    if "nc" not in _CACHE:
        _CACHE["nc"] = build_nc()
    nc1 = _CACHE["nc"]
    ncR = build_nc(reps=reps)
    run1 = _make_runner(nc1, in_maps)
    runR = _make_runner(ncR, in_maps)
    run1(); runR()  # warm-up / compile
    margs = []
    for _ in range(iters):
        t1 = run1()
        tR = runR()
        margs.append((tR - t1) / (reps - 1))
    return margs


# revision 13
# speedup vs baseline: 1.0077x; 1.0077x over previous
"""Trainium2 Bass kernel for CrossShotTransitionHamiltonian.

Math: H = H_idx (x) I_64 with H_idx the 16x16 cycle adjacency matrix, so
U_b = exp(-lam_b H) = M_b (x) I_64 where M_b = expm(-lam_b * H_idx) is a
16x16 symmetric matrix computed exactly on the host from the (tiny) batch
scalars lam_b.  The heavy device work per batch element is the congruence
rho_out = A rho A (A = M (x) I_64, all symmetric) plus trace normalization
(trace folded into the stage-2 operand on the host).

Device algorithm per batch (1024x1024), per core (4 batches/core):
  - rho arrives pre-permuted by the host so partition p = a*16 + k holds
    rows k*64 + a*8 + q and the free dim holds columns in the order
    f = q*1024 + u*128 + blo*32 + (l*2 + bhi)   (c = l*64 + bhi*32 +
    blo*8 + u).  A then acts as a dense 128x128 stationary operand:
    Z = A @ rho (16 matmuls, fp32 PSUM), and every PSUM->SBUF copy is a
    plain contiguous [128,1024] cast copy.
  - ONE DVE stream-transpose instruction (independent 32x32 block
    transposes) moves the contraction index l onto partitions:
    zt[32P + l*2 + bhi, ...].  The 32-block row-group P that lands on the
    contraction axis is neutralized by a modified stage-2 operator
        kron2p[P*32 + l*2 + bhi, j*8 + bhi*4 + P] = M[l,j] / trace
    (block-diagonal over (P, bhi), contracts only l) -- so no PE
    transposes and no strided engine access patterns anywhere.
  - Stage 2: Y = (A/trace) @ Z^T (16 matmuls), contiguous copies, one
    flat contiguous DMA out.  The host un-permutes the result (out is
    symmetric, orientation is irrelevant).

The whole pipeline runs in bf16 (fp32 PSUM accumulation); measured
end-to-end rel err ~3.4e-3 vs the 2e-2 gate.  PSUM->SBUF copies run 2:1
on Activation vs DVE (DVE also owns the stream transposes).

Data-parallel over batch across 8 NeuronCores, no collectives.
"""

import numpy as np

from concourse import bacc, mybir
from concourse import tile
from concourse.bass_utils import run_bass_kernel_spmd

NB = 4  # batch elements per core
NCORES = 8
D = 1024
F32 = mybir.dt.float32
BF16 = mybir.dt.bfloat16


def _build_body(nc, tc, rho_d, kron_d, kron2_d, out_d, nb=NB, reps=1):
    from contextlib import ExitStack

    with ExitStack() as ctx:
        pool = ctx.enter_context(tc.tile_pool(name="work", bufs=1))
        pp = ctx.enter_context(tc.tile_pool(name="ps", bufs=1, space="PSUM"))

        def copy_engine(n):
            # 3:1 rotation ACT:DVE (DVE also runs the stream transposes)
            return nc.scalar.copy if n % 4 != 3 else nc.vector.tensor_copy

        ncopy = 0
        for r in range(reps):
            for i in range(nb):
                u = f"{r}_{i}"
                zin = pool.tile([128, 8192], BF16, tag="zin", bufs=2, name=f"zin{u}")
                nc.sync.dma_start(out=zin[:], in_=rho_d[i])
                kr = pool.tile([128, 128], BF16, tag="kr", bufs=2, name=f"kr{u}")
                nc.scalar.dma_start(out=kr[:], in_=kron_d[i])
                kr2 = pool.tile([128, 128], BF16, tag="kr2", bufs=2, name=f"kr2{u}")
                nc.scalar.dma_start(out=kr2[:], in_=kron2_d[i])

                # ---------- stage 1: Z = A @ rho ----------
                zsb = pool.tile([128, 8192], BF16, tag="zsb", bufs=2, name=f"zsb{u}")
                for q in range(4):
                    pz = pp.tile([128, 2048], F32, tag="pmm", bufs=2, name=f"pz{u}_{q}")
                    for h in range(4):
                        sl = slice(2048 * q + 512 * h, 2048 * q + 512 * (h + 1))
                        nc.tensor.matmul(
                            pz[:, 512 * h : 512 * (h + 1)],
                            lhsT=kr[:],
                            rhs=zin[:, sl],
                            start=True, stop=True,
                        )
                    copy_engine(ncopy)(
                        out=zsb[:, 2048 * q : 2048 * (q + 1)], in_=pz[:]
                    )
                    ncopy += 1

                # ---------- DVE 32x32 block transpose: Z -> zt ----------
                zt = pool.tile([128, 8192], BF16, tag="zt", bufs=2, name=f"zt{u}")
                nc.vector.transpose(out=zt[:, :4096], in_=zsb[:, :4096])
                nc.vector.transpose(out=zt[:, 4096:], in_=zsb[:, 4096:])

                # ---------- stage 2: Y = (A/trace) @ Z^T ----------
                ysb = pool.tile([128, 8192], BF16, tag="ysb", bufs=2, name=f"ysb{u}")
                for t in range(4):
                    po = pp.tile([128, 2048], F32, tag="pmm", bufs=2, name=f"po{u}_{t}")
                    for h in range(4):
                        sl = slice(2048 * t + 512 * h, 2048 * t + 512 * (h + 1))
                        nc.tensor.matmul(
                            po[:, 512 * h : 512 * (h + 1)],
                            lhsT=kr2[:],
                            rhs=zt[:, sl],
                            start=True, stop=True,
                        )
                    copy_engine(ncopy)(
                        out=ysb[:, 2048 * t : 2048 * (t + 1)], in_=po[:]
                    )
                    ncopy += 1

                nc.scalar.dma_start(out=out_d[i], in_=ysb[:])


def build_nc(nb=NB, reps=1):
    nc = bacc.Bacc(
        "TRN2",
        target_bir_lowering=False,
        debug=False,
        enable_asserts=False,
        num_devices=NCORES,
    )
    rho_d = nc.dram_tensor("rho", (nb, 128, 8192), BF16, kind="ExternalInput").ap()
    kron_d = nc.dram_tensor("kron", (nb, 128, 128), BF16, kind="ExternalInput").ap()
    kron2_d = nc.dram_tensor("kron2", (nb, 128, 128), BF16, kind="ExternalInput").ap()
    out_d = nc.dram_tensor("out", (nb, 128, 8192), BF16, kind="ExternalOutput").ap()

    with tile.TileContext(nc) as tc:
        _build_body(nc, tc, rho_d, kron_d, kron2_d, out_d, nb=nb, reps=reps)
    nc.compile()
    return nc


# ---------------- host-side parameter prep ----------------

def _bf16(x):
    import ml_dtypes

    return np.asarray(x, dtype=np.float32).astype(ml_dtypes.bfloat16)


def _index_maps():
    """Flat gather indices: IN maps device rho layout -> natural rho; OUT
    maps device out layout -> natural out positions."""
    if "idx" in _CACHE:
        return _CACHE["idx"]
    # input: position (p=a*16+k, q, f_c=(u,blo,l,bhi)) <- rho[row, col]
    a, k, q = np.meshgrid(np.arange(8), np.arange(16), np.arange(8), indexing="ij")
    row = (k * 64 + a * 8 + q).reshape(128, 8)  # [p, q]
    u2, blo, l, bhi = np.meshgrid(
        np.arange(8), np.arange(4), np.arange(16), np.arange(2), indexing="ij"
    )
    col = (l * 64 + bhi * 32 + blo * 8 + u2).reshape(1024)  # [f_c]
    in_idx = (row[:, :, None] * 1024 + col[None, None, :]).reshape(-1)  # (128*8192,)

    # output: position (y=j*8+bhi*4+P, f=q*1024+u*128+blo*32+v) -> out[c2, r]
    j, bhi2, P = np.meshgrid(np.arange(16), np.arange(2), np.arange(4), indexing="ij")
    j, bhi2, P = j.reshape(128), bhi2.reshape(128), P.reshape(128)  # [y]
    q2, u3, blo2, v = np.meshgrid(
        np.arange(8), np.arange(8), np.arange(4), np.arange(32), indexing="ij"
    )
    q2, u3, blo2, v = (x.reshape(8192) for x in (q2, u3, blo2, v))  # [f]
    c2 = j[:, None] * 64 + bhi2[:, None] * 32 + blo2[None, :] * 8 + u3[None, :]
    rr = P[:, None] * 256 + v[None, :] * 8 + q2[None, :]
    out_idx = (c2 * 1024 + rr).reshape(-1)  # (128*8192,)
    _CACHE["idx"] = (in_idx, out_idx)
    return in_idx, out_idx


def _host_params(t, w1, b1, w2, b2):
    x = t.astype(np.float64)[:, None]
    h = x @ w1.astype(np.float64).T + b1.astype(np.float64)
    h = h / (1.0 + np.exp(-h))  # silu
    lam = 0.1 * np.tanh(h @ w2.astype(np.float64).T + b2.astype(np.float64))[:, 0]

    k = np.arange(16)
    S = np.zeros((16, 16))
    S[(k + 1) % 16, k] = 1.0
    Hidx = S + S.T
    w_eig, V = np.linalg.eigh(Hidx)
    E = np.exp(-lam[:, None] * w_eig[None, :])  # (B,16)
    M = np.einsum("ik,bk,jk->bij", V, E, V)  # (B,16,16)

    B = M.shape[0]
    # stage-1 lhsT: in-partitions a-major (p = a*16 + k), out (m = i*8 + a):
    # kron1[b, a*16+k, i*8+a] = M[b, i, k]
    kron1 = np.zeros((B, 8, 16, 16, 8))
    for a_sub in range(8):
        kron1[:, a_sub, :, :, a_sub] = np.transpose(M, (0, 2, 1))
    kron = np.ascontiguousarray(kron1.reshape(B, 128, 128), dtype=np.float32)
    # stage-2 lhsT for the 32x32-block-transposed zt:
    # kron2p[b, P*32 + l*2 + bhi, j*8 + bhi*4 + P] = M[b, l, j]
    kron2p = np.zeros((B, 4, 16, 2, 16, 2, 4))
    for P in range(4):
        for bhi in range(2):
            kron2p[:, P, :, bhi, :, bhi, P] = M
    kron2 = np.ascontiguousarray(kron2p.reshape(B, 128, 128), dtype=np.float32)
    return kron, kron2


_CACHE = {}


def _host_traces(rho, t, w1, b1, w2, b2):
    """tr(A^2 rho) per batch from rho's block diagonals (tiny: 0.5M MACs)."""
    x = t.astype(np.float64)[:, None]
    h = x @ w1.astype(np.float64).T + b1.astype(np.float64)
    h = h / (1.0 + np.exp(-h))
    lam = 0.1 * np.tanh(h @ w2.astype(np.float64).T + b2.astype(np.float64))[:, 0]
    k = np.arange(16)
    S = np.zeros((16, 16))
    S[(k + 1) % 16, k] = 1.0
    w_eig, V = np.linalg.eigh(S + S.T)
    E = np.exp(-lam[:, None] * w_eig[None, :])
    M = np.einsum("ik,bk,jk->bij", V, E, V)
    M2 = np.einsum("bij,bjk->bik", M, M)
    rr = rho.reshape(rho.shape[0], 16, 64, 16, 64)
    c = np.einsum("bkala->bkl", rr, optimize=True)
    return np.einsum("bkl,bkl->b", c.astype(np.float64), M2)


def _prep_in_maps(rho, t, w1, b1, w2, b2):
    rho = np.ascontiguousarray(rho, dtype=np.float32)
    kron, kron2 = _host_params(
        np.asarray(t), np.asarray(w1), np.asarray(b1), np.asarray(w2), np.asarray(b2)
    )
    tr = _host_traces(rho, np.asarray(t), np.asarray(w1), np.asarray(b1),
                      np.asarray(w2), np.asarray(b2))
    kron2 = kron2 / np.maximum(tr, 1e-8)[:, None, None]
    in_idx, _ = _index_maps()
    B = rho.shape[0]
    rho_p = np.take(rho.reshape(B, -1), in_idx, axis=1).reshape(B, 128, 8192)
    rho_b = _bf16(rho_p)
    kron_b = _bf16(kron)
    kron2_b = _bf16(kron2)

    in_maps = []
    for c in range(NCORES):
        sl = slice(NB * c, NB * (c + 1))
        in_maps.append(
            {
                "rho": np.ascontiguousarray(rho_b[sl]),
                "kron": np.ascontiguousarray(kron_b[sl]),
                "kron2": np.ascontiguousarray(kron2_b[sl]),
            }
        )
    return in_maps


def kernel(rho, t, w1, b1, w2, b2, H):
    in_maps = _prep_in_maps(rho, t, w1, b1, w2, b2)
    if "nc" not in _CACHE:
        _CACHE["nc"] = build_nc()
    nc = _CACHE["nc"]

    last_err = None
    for attempt in range(3):
        try:
            res = run_bass_kernel_spmd(nc, in_maps, core_ids=list(range(NCORES)))
            break
        except Exception as e:  # transient device-unrecoverable faults heal on retry
            last_err = e
            import time as _time

            _time.sleep(5.0)
    else:
        raise last_err
    dev = np.concatenate(
        [np.asarray(res.results[c]["out"], dtype=np.float32) for c in range(NCORES)],
        axis=0,
    )
    _, out_idx = _index_maps()
    B = dev.shape[0]
    out = np.empty((B, D * D), dtype=np.float32)
    out[:, out_idx] = dev.reshape(B, -1)
    return out.reshape(B, D, D)


def _make_runner(nc, in_maps):
    """Build a jitted dispatcher for nc; returns fn() -> wall ns per call."""
    import time
    import jax
    import jax.numpy as jnp
    from jax.experimental.shard_map import shard_map
    from jax.sharding import Mesh, NamedSharding, PartitionSpec

    from concourse.bass2jax import _bass_exec_p, install_neuronx_cc_hook
    from concourse.bass2jax import partition_id_tensor

    install_neuronx_cc_hook()
    part_name = nc.partition_id_tensor.name if nc.partition_id_tensor else None
    in_names, out_names, out_avals, zero_outs = [], [], [], []
    for alloc in nc.m.functions[0].allocations:
        if not isinstance(alloc, mybir.MemoryLocationSet):
            continue
        name = alloc.memorylocations[0].name
        if alloc.kind == "ExternalInput":
            if name != part_name:
                in_names.append(name)
        elif alloc.kind == "ExternalOutput":
            out_names.append(name)
            shape = tuple(alloc.tensor_shape)
            dtype = mybir.dt.np(alloc.dtype)
            out_avals.append(jax.core.ShapedArray(shape, dtype))
            zero_outs.append((shape, dtype))
    n_params = len(in_names)
    n_outs = len(out_avals)
    all_names = in_names + out_names
    if part_name is not None:
        all_names = all_names + [part_name]
    donate = tuple(range(n_params, n_params + n_outs))

    def _body(*args):
        operands = list(args)
        if part_name is not None:
            operands.append(partition_id_tensor())
        outs = _bass_exec_p.bind(
            *operands,
            out_avals=tuple(out_avals),
            in_names=tuple(all_names),
            out_names=tuple(out_names),
            lowering_input_output_aliases=(),
            sim_require_finite=True,
            sim_require_nnan=True,
            nc=nc,
        )
        return tuple(outs)

    devices = jax.devices()[:NCORES]
    mesh = Mesh(np.asarray(devices), ("core",))
    in_specs = (PartitionSpec("core"),) * (n_params + n_outs)
    out_specs = (PartitionSpec("core"),) * n_outs
    sharded = jax.jit(
        shard_map(_body, mesh=mesh, in_specs=in_specs, out_specs=out_specs,
                  check_rep=False),
        donate_argnums=donate,
        keep_unused=True,
    )
    sh = NamedSharding(mesh, PartitionSpec("core"))
    concat_in = [
        jax.device_put(
            np.concatenate([np.asarray(in_maps[c][n])[None] for c in range(NCORES)],
                           axis=0).reshape((-1, *np.asarray(in_maps[0][n]).shape[1:])),
            sh,
        )
        for n in in_names
    ]
    mkz = jax.jit(
        lambda: tuple(
            jnp.zeros((NCORES * s[0], *s[1:]), d) for (s, d) in zero_outs
        ),
        out_shardings=tuple(sh for _ in zero_outs),
    )

    def run():
        zs = mkz()
        jax.block_until_ready(zs)
        t0 = time.perf_counter()
        out = sharded(*concat_in, *zs)
        jax.block_until_ready(out)
        t1 = time.perf_counter()
        return (t1 - t0) * 1e9

    return run


def timed_runs(inputs, iters=10, nc=None):
    """Wall times (ns) for repeated dispatches of one NEFF."""
    in_maps = _prep_in_maps(
        inputs["rho"], inputs["t"], inputs["w1"], inputs["b1"],
        inputs["w2"], inputs["b2"],
    )
    if nc is None:
        if "nc" not in _CACHE:
            _CACHE["nc"] = build_nc()
        nc = _CACHE["nc"]
    run = _make_runner(nc, in_maps)
    run()  # warm-up / compile
    return [run() for _ in range(iters)]


def timed_pairs(inputs, reps, iters=14):
    """Interleave 1-rep and reps-rep NEFF dispatches; the paired difference
    cancels the slow drift of the RPC dispatch floor.  Returns per-pair
    marginal device ns/exec list: (tR - t1) / (reps - 1)."""
    in_maps = _prep_in_maps(
        inputs["rho"], inputs["t"], inputs["w1"], inputs["b1"],
        inputs["w2"], inputs["b2"],
    )
    if "nc" not in _CACHE:
        _CACHE["nc"] = build_nc()
    nc1 = _CACHE["nc"]
    ncR = build_nc(reps=reps)
    run1 = _make_runner(nc1, in_maps)
    runR = _make_runner(ncR, in_maps)
    run1(); runR()  # warm-up / compile
    margs = []
    for _ in range(iters):
        t1 = run1()
        tR = runR()
        margs.append((tR - t1) / (reps - 1))
    return margs
